# revision 51
# baseline (speedup 1.0000x reference)
"""Trainium2 Bass kernel for nn_Attention_25915832664752.

Reference computation (per reference.py):
    For b in {Q,K,V}:  q0 = relu(IN(conv1d(Z, W[b,0])));  q1 = relu(IN(conv1d(Z, W[b,1]) @ L))
                       X_b = q0 + q1                                  [2048, 48]
    A  = exp(Q @ K^T)                                                 [2048, 2048]
    P  = A / rowsum(A);  Aa = (P + P^T)/2;  out = Aa @ V              [2048, 48]

Strategy (8 NeuronCores, tensor-parallel over nhid):
    Core c owns output channels [c*256, (c+1)*256).  W is pre-transposed on the
    host into a per-core streaming slab Wt[kt, p, o] with contraction index
    k on the partition axis, so the conv becomes a pure stream of [128, 48]
    stationary (shifted Z window) x W^T-column moving matmuls accumulated in
    PSUM — W (56 MB/core bf16) is read from HBM exactly once at full rate.
    BOTH conv branches share the same Z-window stationary: the "@ L" of the
    r=1 branch is applied at the epilogue, fused into the PSUM->IN-layout
    transpose (multiply by L instead of the identity — zero extra matmuls).
    After the convs: instance-norm + relu fused into one scalar-engine
    activation per tile; K and V are all-gathered; each core computes its
    row-block A_loc = exp(Q_loc K_full^T) and the transposed block
    At = exp(K_full Q_loc^T); then
        out = 0.5*rinv*(At^T-contract V_full) + 0.5*ReduceScatter(A_loc^T
              row-scaled V_loc)
    which realizes the symmetrized row-normalized attention exactly; the
    At/term1 compute hides under the ReduceScatter.
"""

import os
import sys

import numpy as np

sys.path.insert(0, "/opt/trn_rl_repo")

import orjson

import concourse.bass as bass
import concourse.mybir as mybir
from concourse import masks, tile
from concourse.bass_utils import run_bass_kernel_spmd

# ---------------------------------------------------------------- waitfix ---
# This neuronxcc build allows only ONE sync wait per instruction;
# TileContext emits instructions with several.  Rewrite the serialized BIR:
# hoist extra waits onto standalone NoOps inserted just before the
# instruction on the same engine (cumulative thresholds -> semantics kept).

_DMA_OPCODES = {
    "DMACopy", "DMATranspose", "TensorLoad", "TensorSave",
    "TriggeredCopy", "CollectiveCompute",
}
_wfix_counter = [0]


def _fix_block(instructions):
    out = []
    for ins in instructions:
        si = ins.get("sync_info")
        if not si:
            out.append(ins)
            continue
        waits = si.get("on_wait") or []
        updates = si.get("on_update") or []
        if len(waits) > 1:
            for w in waits[1:]:
                _wfix_counter[0] += 1
                out.append({
                    "engine": ins["engine"], "ins": [],
                    "name": f"WFIX-{_wfix_counter[0]}", "opcode": "NoOp",
                    "outs": [],
                    "sync_info": {"on_update": [], "on_wait": [w]},
                })
            si["on_wait"] = waits[:1]
        deferred = []
        if len(updates) > 1:
            assert ins.get("opcode", "") not in _DMA_OPCODES, (
                f"multi-update on DMA opcode: {ins['name']}"
            )
            si["on_update"] = updates[:1]
            for u in updates[1:]:
                _wfix_counter[0] += 1
                deferred.append({
                    "engine": ins["engine"], "ins": [],
                    "name": f"WFIX-{_wfix_counter[0]}", "opcode": "NoOp",
                    "outs": [],
                    "sync_info": {"on_update": [u], "on_wait": []},
                })
        out.append(ins)
        out.extend(deferred)
    return out


def _fix_bir_json_bytes(data: bytes) -> bytes:
    d = orjson.loads(data)
    for func in d.get("functions", []):
        for bb in func.get("blocks", []):
            bb["instructions"] = _fix_block(bb["instructions"])
    return orjson.dumps(d)


if not getattr(bass.Bass, "_waitfix_installed", False):
    _orig_to_json_bytes = bass.Bass.to_json_bytes

    def _patched_to_json_bytes(self) -> bytes:
        return _fix_bir_json_bytes(_orig_to_json_bytes(self))

    bass.Bass.to_json_bytes = _patched_to_json_bytes
    bass.Bass._waitfix_installed = True

# Synthesize the missing ``antenv.axon_hooks`` module so that
# ``run_bass_kernel_spmd(trace=True)`` can drive NTFF profiling through the
# axon PJRT plugin (the boot-time registration degrades silently when the
# module is absent).  Harmless when tracing is never requested.
try:
    import types

    import antenv

    if not hasattr(antenv, "axon_hooks"):
        _hooks_mod = types.ModuleType("antenv.axon_hooks")
        _ntff_hook = [None]
        _hooks_mod.set_axon_ntff_profile_hook = lambda h: _ntff_hook.__setitem__(0, h)
        _hooks_mod.get_axon_ntff_profile_hook = lambda: _ntff_hook[0]
        sys.modules["antenv.axon_hooks"] = _hooks_mod
        antenv.axon_hooks = _hooks_mod
        from trn_agent_boot.trn_boot import _ntff_profile_via_ctypes

        _hooks_mod.set_axon_ntff_profile_hook(
            _ntff_profile_via_ctypes("/opt/axon/libaxon_pjrt.so"))

    import concourse.bass_utils as _bu

    _bu.upload_artifacts = lambda tmpdir: tmpdir  # no fish share in container
except Exception:  # pragma: no cover - profiling is best-effort
    pass

# ------------------------------------------------------------- constants ---

NHID = 2048
NOPEN = 2048
N = 48          # spatial length
KD = 9          # conv kernel width
PAD = 4
NP = N + 2 * PAD            # 56 padded spatial
EPS = 1e-5
CORES = 8
OLOC = NHID // CORES        # 256 output channels per core
NGRP = 6                    # (b, r) conv groups
OCOLS = NGRP * OLOC         # 1536 W^T columns per core
KTOT = KD * NOPEN           # 18432 contraction length
NKT = KTOT // 128           # 144 k-tiles
ISUB = NOPEN // 128         # 16 i-subtiles
CKA = 2                     # k-tiles per W DMA chunk (sweep A, 0.5 MB bf16)
NCHA = NKT // CKA           # 72 chunks (sweep A)
CKB = 8                     # k-tiles per W DMA chunk (sweep B, 1 MB bf16)
NCHB = NKT // CKB           # 18 chunks (sweep B)
ACOLS = 4 * OLOC            # sweep A (Q,K): 1024 W^T cols per k-row
BCOLS = 2 * OLOC            # sweep B (V):    512 W^T cols per k-row
F32 = mybir.dt.float32
F32R = mybir.dt.float32r
BF16 = mybir.dt.bfloat16


DEBUG = bool(int(os.environ.get("KERNEL_DEBUG", "0")))


def _build_nc():
    nc = bass.Bass()

    wta_d = nc.declare_dram_parameter(
        "wta", [NCHA, 128, CKA * ACOLS], BF16, isOutput=False)
    wtb_d = nc.declare_dram_parameter(
        "wtb", [NCHB, 128, CKB * BCOLS], BF16, isOutput=False)
    z_d = nc.declare_dram_parameter("z", [NOPEN, N], F32, isOutput=False)
    l_d = nc.declare_dram_parameter("l", [N, N], F32, isOutput=False)
    out_d = nc.declare_dram_parameter("out", [N, OLOC], F32, isOutput=True)
    if DEBUG:
        dbg_conv = nc.declare_dram_parameter(
            "dbg_conv", [12, 128, N], F32, isOutput=True)
        dbg_qkv = nc.declare_dram_parameter(
            "dbg_qkv", [3, 2, 128, N], F32, isOutput=True)
        dbg_rs = nc.declare_dram_parameter(
            "dbg_rs", [2, 128, 1], F32, isOutput=True)

    with tile.TileContext(nc) as tc:
        with (
            tc.tile_pool(name="pers", bufs=1) as pers,
            tc.tile_pool(name="wpool", bufs=8) as wpool,
            tc.tile_pool(name="wpoolb", bufs=5) as wpoolb,
            tc.tile_pool(name="stats", bufs=1) as stats,
            tc.tile_pool(name="pacc", bufs=2, space="PSUM") as pacc,
            tc.tile_pool(name="ptrans", bufs=2, space="PSUM") as ptrans,
            tc.tile_pool(name="dram", bufs=1, space="DRAM") as dram,
        ):
            # ---------------- prologue: Z, L, identity, ZpadT, ZcolL -------
            ident = pers.tile([128, 128], F32, tag="ident")
            masks.make_identity(nc, ident[:])
            ident16 = pers.tile([128, 128], BF16, tag="ident16")
            nc.vector.tensor_copy(ident16[:], ident[:])

            # preload the Exp activation table now so the mid-kernel EXP
            # doesn't eat an ACT_TABLE_LOAD on the critical tail
            warm = pers.tile([128, 1], F32, tag="warm")
            nc.scalar.activation(warm[0:1, 0:1], ident[0:1, 0:1],
                                 mybir.ActivationFunctionType.Exp)
            ones1 = pers.tile([128, N], F32, tag="ones1")
            nc.vector.memset(ones1[0:1, :], 1.0)

            rg = [list(range(CORES))]

            # First two W chunks DMA before anything else so the conv can
            # start the moment zpadr is ready
            wts_pre = {}
            for g in (0, 1):
                wt0 = wpool.tile([128, CKA * ACOLS], BF16, tag="wt",
                                 name=f"wta{g}")
                nc.sync.dma_start(out=wt0[:], in_=wta_d[g])
                wts_pre[g] = wt0

            # Z loaded contiguously (channel i = p*16 + a: one 3 KB
            # descriptor per partition; _prep_w permutes W's k-rows to
            # match), then ONE strided vector cast-copy into the padded
            # bf16 conv-stationary layout: 16 tiles [128, 56] side by side.
            ztmp = pers.tile([128, ISUB * N], F32, tag="ztmp")
            nc.sync.dma_start(
                out=ztmp[:],
                in_=z_d[:].rearrange("(p a) n -> p (a n)", p=128))

            # L [48, 48] — used at the conv epilogues to apply the r=1
            # branch's "@ L" fused with the IN-layout transpose
            l_sb = pers.tile([128, N], F32, tag="l_sb")
            nc.sync.dma_start(out=l_sb[0:N, :], in_=l_d[:])

            zpadr = pers.tile([128, ISUB * NP], BF16, tag="zpadr")
            nc.vector.memset(zpadr[:], 0.0)
            zpr_v = zpadr[:].rearrange("p (a c) -> p a c", c=NP)
            nc.vector.tensor_copy(
                zpr_v[:, :, PAD:PAD + N],
                ztmp[:].rearrange("p (a n) -> p a n", n=N))

            # ---------------- conv: stream W as the MOVING operand ---------
            # lhsT (stationary) = [128, 48] shifted Z window, shared by BOTH
            # branch accumulators; rhs = W^T columns streaming at 1 col/cycle.
            # Two k-sweeps: A covers K+V groups, B covers Q, so the K/V
            # all-gathers hide behind sweep B.  One PSUM bank per branch
            # accumulator (start=True clears has_written for the whole bank).
            relu_sc = pers.tile([128, 12 * N], F32, tag="relu_sc")
            yt_sb = pers.tile([128, 6 * OLOC], F32, tag="yt_sb")
            qkv = [pers.tile([128, 2 * N], F32, tag=f"qkv{b}", name=f"qkv{b}")
                   for b in range(3)]
            slotinfo = {}

            def sweep_epilogue(entries, label):
                """entries: list of (g, acc_ap[48, 256], is_r1).  Transpose
                each half to [128, 48] — for r=1 groups multiply by L instead
                of the identity, realizing (conv @ L)^T in the same matmul —
                then batched instance-norm stats (one vector op per stage
                across all slots) + fused relu."""
                nslot = 2 * len(entries)
                xc = stats.tile([128, nslot * N], F32, tag=f"xc{label}",
                                name=f"xc{label}")
                slots = []
                for idx, (g, acc_ap, is_r1) in enumerate(entries):
                    nc.scalar.copy(
                        yt_sb[0:N, g * OLOC:(g + 1) * OLOC], acc_ap)
                    rmat = l_sb if is_r1 else ident
                    for h in range(2):
                        ot = g * 2 + h
                        slot = idx * 2 + h
                        ps2 = ptrans.tile([128, 128], F32, tag="ptrans",
                                          name=f"tp{ot}")
                        nc.tensor.matmul(
                            ps2[:, 0:N],
                            yt_sb[0:N, g * OLOC + h * 128:
                                  g * OLOC + (h + 1) * 128],
                            rmat[0:N, 0:N],
                            start=True, stop=True)
                        nc.scalar.copy(xc[:, slot * N:(slot + 1) * N],
                                       ps2[:, 0:N])
                        if DEBUG:
                            nc.scalar.dma_start(
                                out=dbg_conv[ot],
                                in_=xc[:, slot * N:(slot + 1) * N])
                        slots.append((ot, slot))
                sm = stats.tile([128, nslot], F32, tag=f"sm{label}",
                                name=f"sm{label}")
                sq = stats.tile([128, nslot], F32, tag=f"sq{label}",
                                name=f"sq{label}")
                scr = stats.tile([128, nslot * N], F32, tag=f"scr{label}",
                                 name=f"scr{label}")
                for ot, slot in slots:
                    nc.vector.reduce_sum(
                        sm[:, slot:slot + 1], xc[:, slot * N:(slot + 1) * N],
                        axis=mybir.AxisListType.X)
                nc.vector.tensor_tensor(scr[:], xc[:], xc[:],
                                        op=mybir.AluOpType.mult)
                for ot, slot in slots:
                    nc.vector.reduce_sum(
                        sq[:, slot:slot + 1], scr[:, slot * N:(slot + 1) * N],
                        axis=mybir.AxisListType.X)
                mean = stats.tile([128, nslot], F32, tag=f"mean{label}",
                                  name=f"mean{label}")
                var = stats.tile([128, nslot], F32, tag=f"var{label}",
                                 name=f"var{label}")
                std = stats.tile([128, nslot], F32, tag=f"std{label}",
                                 name=f"std{label}")
                rsv = stats.tile([128, nslot], F32, tag=f"rsv{label}",
                                 name=f"rsv{label}")
                nc.vector.tensor_scalar_mul(mean[:], sm[:], 1.0 / N)
                nc.vector.tensor_scalar_mul(sq[:], sq[:], 1.0 / N)
                nc.vector.tensor_tensor(var[:], mean[:], mean[:],
                                        op=mybir.AluOpType.mult)
                nc.vector.tensor_tensor(var[:], sq[:], var[:],
                                        op=mybir.AluOpType.subtract)
                nc.vector.tensor_scalar_add(var[:], var[:], EPS)
                nc.scalar.sqrt(std[:], var[:])
                nc.vector.reciprocal(rsv[:], std[:])
                if label == "B":
                    # preload the Exp table while the vector engine runs the
                    # IN+relu tail, so the attention EXPs start cold-free
                    nc.scalar.activation(warm[0:1, 0:1], ident[0:1, 0:1],
                                         mybir.ActivationFunctionType.Exp)
                for ot, slot in slots:
                    slotinfo[ot] = (xc, scr, slot, mean, rsv)

            def qkv_add(b):
                # IN + relu + branch-add fused on the vector engine:
                #   q = rsv0*max(x0-m0, 0) + rsv1*max(x1-m1, 0)
                for h in range(2):
                    ot0 = (2 * b) * 2 + h        # r = 0
                    ot1 = (2 * b + 1) * 2 + h    # r = 1
                    xc0, scr0, s0, mean0, rsv0 = slotinfo[ot0]
                    xc1, scr1, s1, mean1, rsv1 = slotinfo[ot1]
                    nc.vector.tensor_scalar(
                        scr0[:, s0 * N:(s0 + 1) * N],
                        xc0[:, s0 * N:(s0 + 1) * N],
                        mean0[:, s0:s0 + 1], 0.0,
                        op0=mybir.AluOpType.subtract, op1=mybir.AluOpType.max)
                    nc.vector.tensor_scalar(
                        scr1[:, s1 * N:(s1 + 1) * N],
                        xc1[:, s1 * N:(s1 + 1) * N],
                        mean1[:, s1:s1 + 1], 0.0,
                        op0=mybir.AluOpType.subtract, op1=mybir.AluOpType.max)
                    nc.vector.tensor_scalar_mul(
                        relu_sc[:, ot1 * N:(ot1 + 1) * N],
                        scr1[:, s1 * N:(s1 + 1) * N], rsv1[:, s1:s1 + 1])
                    nc.vector.scalar_tensor_tensor(
                        out=qkv[b][:, h * N:(h + 1) * N],
                        in0=scr0[:, s0 * N:(s0 + 1) * N],
                        scalar=rsv0[:, s0:s0 + 1],
                        in1=relu_sc[:, ot1 * N:(ot1 + 1) * N],
                        op0=mybir.AluOpType.mult,
                        op1=mybir.AluOpType.add)
                    if DEBUG:
                        nc.scalar.dma_start(
                            out=dbg_qkv[b, h],
                            in_=qkv[b][:, h * N:(h + 1) * N])

            # ---- sweep A: K + V (cols [g2,g4 | g3,g5]; both branch
            # accumulators share one Z-window stationary per k-tile)
            accA = [pacc.tile([128, 2 * OLOC], F32, tag="accw", name=f"accA{i}")
                    for i in range(2)]
            for gch in range(NCHA):
                if gch in wts_pre:
                    wt = wts_pre.pop(gch)
                else:
                    wt = wpool.tile([128, CKA * ACOLS], BF16, tag="wt",
                                    name=f"wta{gch}")
                    nc.sync.dma_start(out=wt[:], in_=wta_d[gch])
                for j in range(CKA):
                    kt = gch * CKA + j
                    t, s = kt // ISUB, kt % ISUB
                    lhs0 = zpadr[:, s * NP + t: s * NP + t + N]
                    base = j * ACOLS
                    nc.tensor.matmul(
                        accA[0][0:N, :], lhs0, wt[:, base: base + 512],
                        start=(kt == 0), stop=(kt == NKT - 1))
                    nc.tensor.matmul(
                        accA[1][0:N, :], lhs0, wt[:, base + 512: base + 1024],
                        start=(kt == 0), stop=(kt == NKT - 1))

            # K then V epilogues; ONE merged all-gather of [256, 96]
            # (K | V side by side) hides behind sweep B (Q)
            qloc, kloc, vloc = qkv
            kvb = dram.tile([OLOC, 2 * N], BF16, tag="kvb")
            kvg = dram.tile([NHID, 2 * N], BF16, tag="kvg",
                            addr_space="Shared")
            kv16 = pers.tile([128, 4 * N], BF16, tag="kv16")

            sweep_epilogue([(2, accA[0][0:N, 0:OLOC], False),
                            (3, accA[1][0:N, 0:OLOC], True)], "K")
            qkv_add(1)
            nc.vector.tensor_copy(kv16[:, 0:2 * N], kloc[:])

            sweep_epilogue([(4, accA[0][0:N, OLOC:2 * OLOC], False),
                            (5, accA[1][0:N, OLOC:2 * OLOC], True)], "V")
            qkv_add(2)
            nc.vector.tensor_copy(kv16[:, 2 * N:4 * N], vloc[:])
            # kv16 col blocks are [K0 K1 V0 V1]; kvb row h*128+p gets
            # [K_h | V_h] for channel h*128+p
            nc.scalar.dma_start(
                out=kvb[:, 0:N].rearrange("(h p) n -> p h n", h=2),
                in_=kv16[:, 0:2 * N].rearrange("p (h n) -> p h n", h=2))
            nc.scalar.dma_start(
                out=kvb[:, N:2 * N].rearrange("(h p) n -> p h n", h=2),
                in_=kv16[:, 2 * N:4 * N].rearrange("p (h n) -> p h n", h=2))
            nc.gpsimd.collective_compute(
                "AllGather", mybir.AluOpType.bypass,
                replica_groups=rg, ins=[kvb.opt()], outs=[kvg.opt()])
            kvfull = pers.tile([128, 16 * 2 * N], BF16, tag="kvfull")
            nc.gpsimd.dma_start(
                out=kvfull[:].rearrange("p (a c) -> p a c", c=2 * N),
                in_=kvg[:].rearrange("(a p) c -> p a c", p=128))

            # ---- sweep B: Q ----
            accB = [pacc.tile([128, OLOC], F32, tag="acc", name=f"accB{i}")
                    for i in range(2)]  # order: g0, g1

            def sweep_b(c0, c1):
                for gch in range(c0, c1):
                    wt = wpoolb.tile([128, CKB * BCOLS], BF16, tag="wtb",
                                     name=f"wtb{gch}")
                    nc.sync.dma_start(out=wt[:], in_=wtb_d[gch])
                    for j in range(CKB):
                        kt = gch * CKB + j
                        t, s = kt // ISUB, kt % ISUB
                        lhs0 = zpadr[:, s * NP + t: s * NP + t + N]
                        base = j * BCOLS
                        nc.tensor.matmul(
                            accB[0][0:N, :], lhs0,
                            wt[:, base: base + OLOC],
                            start=(kt == 0), stop=(kt == NKT - 1))
                        nc.tensor.matmul(
                            accB[1][0:N, :], lhs0,
                            wt[:, base + OLOC: base + 2 * OLOC],
                            start=(kt == 0), stop=(kt == NKT - 1))

            sweep_b(0, NCHB - 2)

            # kT transposes tucked into sweep B's DMA-bound PE headroom
            # (the merged AG lands reliably ~15 us before this point)
            kT = pers.tile([128, NHID], BF16, tag="kT")
            for jt in range(16):
                ps = ptrans.tile([128, 128], F32, tag="ptrans")
                nc.tensor.matmul(
                    ps[0:N, :], kvfull[:, jt * 2 * N:jt * 2 * N + N],
                    ident16[:], start=True, stop=True)
                kt_copy = (nc.scalar.copy if jt % 2 == 0
                           else nc.vector.tensor_copy)
                kt_copy(kT[0:N, jt * 128:(jt + 1) * 128], ps[0:N, :])

            sweep_b(NCHB - 2, NCHB)

            sweep_epilogue([(0, accB[0][0:N, :], False),
                            (1, accB[1][0:N, :], True)], "B")
            qkv_add(0)

            # qT split exactly into bf16 high + low parts: every attention
            # matmul runs with bf16 operands at full PE rate with no
            # precision loss vs f32 Q (K is bf16-limited by the all-gather).
            qTh = pers.tile([128, 2 * 128], BF16, tag="qTh")
            qTl = pers.tile([128, 2 * 128], BF16, tag="qTl")
            for h in range(2):
                ps = ptrans.tile([128, 128], F32, tag="ptrans")
                nc.tensor.transpose(
                    ps[0:N, :], qloc[:, h * N:(h + 1) * N], ident[:])
                nc.scalar.copy(qTh[0:N, h * 128:(h + 1) * 128], ps[0:N, :])
                nc.vector.tensor_tensor(
                    qTl[0:N, h * 128:(h + 1) * 128], ps[0:N, :],
                    qTh[0:N, h * 128:(h + 1) * 128],
                    op=mybir.AluOpType.subtract)

            # A = exp(Q K^T) chunks (kT already transposed mid-sweep)
            a_sb = [pers.tile([128, NHID], BF16, tag=f"a{m}", name=f"a{m}")
                    for m in range(2)]
            rsparts = [stats.tile([128, 4], F32, tag=f"rsp{m}", name=f"rsp{m}")
                       for m in range(2)]
            for jc in range(4):
                for m in range(2):
                    ps = ptrans.tile([128, 512], F32, tag="pattn",
                                     name=f"pa{m}{jc}", bufs=2)
                    nc.tensor.matmul(
                        ps[:, 0:512],
                        qTh[0:N, m * 128:(m + 1) * 128],
                        kT[0:N, jc * 512:(jc + 1) * 512],
                        start=True, stop=False)
                    nc.tensor.matmul(
                        ps[:, 0:512],
                        qTl[0:N, m * 128:(m + 1) * 128],
                        kT[0:N, jc * 512:(jc + 1) * 512],
                        start=False, stop=True)
                    nc.scalar.activation(
                        a_sb[m][:, jc * 512:(jc + 1) * 512], ps[:, 0:512],
                        mybir.ActivationFunctionType.Exp,
                        accum_out=rsparts[m][:, jc:jc + 1])
            rinvh = []
            for m in range(2):
                rowsum = stats.tile([128, 1], F32, tag=f"rowsum{m}", name=f"rowsum{m}")
                nc.vector.reduce_sum(rowsum[:], rsparts[m][:], axis=mybir.AxisListType.X)
                rinv = stats.tile([128, 1], F32, tag=f"rinv{m}", name=f"rinv{m}")
                nc.vector.reciprocal(rinv[:], rowsum[:])
                rh = stats.tile([128, 1], F32, tag=f"rinvh{m}", name=f"rinvh{m}")
                nc.vector.tensor_scalar_mul(rh[:], rinv[:], 0.5)
                rinvh.append((rinv, rh))
                if DEBUG:
                    nc.scalar.dma_start(out=dbg_rs[m], in_=rowsum[:])

            # ---------------- U^T = (rinv*V_loc)^T-contract A_loc ----------
            # two wide matmuls per 512-col chunk (vr stationary, bf16 a_sb
            # moving) replace 32 LDWEIGHTS-bound small matmuls; then PE
            # transposes back to [2048, 48] for the ReduceScatter.
            vrb = pers.tile([128, 2 * N], BF16, tag="vrb")
            for m in range(2):
                nc.vector.tensor_scalar_mul(
                    vrb[:, m * N:(m + 1) * N], vloc[:, m * N:(m + 1) * N],
                    rinvh[m][0][:])
            uT = pers.tile([128, NHID], BF16, tag="uT")
            for jc in range(4):
                ps = ptrans.tile([128, 512], F32, tag="pattn",
                                 name=f"pu{jc}", bufs=2)
                for m in range(2):
                    nc.tensor.matmul(
                        ps[0:N, :], vrb[:, m * N:(m + 1) * N],
                        a_sb[m][:, jc * 512:(jc + 1) * 512],
                        start=(m == 0), stop=(m == 1))
                nc.scalar.copy(uT[0:N, jc * 512:(jc + 1) * 512], ps[0:N, :])

            # U^T goes to the ReduceScatter in transposed block layout
            # [8, 48, 256] (block c = core c's channel slab): no PE
            # transposes before the collective; each core transposes only
            # its own [48, 256] result afterwards.  Eight contiguous
            # per-block DMAs spread across the engine queues (one strided
            # DMA would cost 384 scattered descriptors on one queue).
            ubt = dram.tile([CORES * N, OLOC], BF16, tag="ubt")
            rsbt = dram.tile([N, OLOC], BF16, tag="rsbt")
            ub_eng = [nc.sync, nc.scalar, nc.gpsimd]
            for c in range(CORES):
                ub_eng[c % 3].dma_start(
                    out=ubt[c * N:(c + 1) * N, :],
                    in_=uT[0:N, c * OLOC:(c + 1) * OLOC])
            nc.gpsimd.collective_compute(
                "ReduceScatter", mybir.AluOpType.add,
                replica_groups=rg, ins=[ubt.opt()], outs=[rsbt.opt()])

            # At = exp(K Q^T)  [2048, 256] — computed while the RS is on
            # the wire (bf16 kT stationary, bf16 qTh/qTl moving)
            at_sb = pers.tile([128, 16 * 256], BF16, tag="at")
            for jt in range(16):
                ps = ptrans.tile([128, 256], F32, tag="ptrans")
                nc.tensor.matmul(
                    ps[:, 0:256],
                    kT[0:N, jt * 128:(jt + 1) * 128],
                    qTh[0:N, 0:256],
                    start=True, stop=False)
                nc.tensor.matmul(
                    ps[:, 0:256],
                    kT[0:N, jt * 128:(jt + 1) * 128],
                    qTl[0:N, 0:256],
                    start=False, stop=True)
                nc.scalar.activation(
                    at_sb[:, jt * 256:(jt + 1) * 256], ps[:, 0:256],
                    mybir.ActivationFunctionType.Exp)

            # --------- term1^T = V_full^T-contract At (one accumulator),
            # pre-scaled by a PE-broadcast 0.5*rinv row — all while the RS
            # is on the wire.  The whole finale stays in [48, 256] layout;
            # the host transposes the gathered output once.
            psT1 = pacc.tile([128, OLOC], F32, tag="acc", name="pt1")
            for jt in range(16):
                nc.tensor.matmul(
                    psT1[0:N, :],
                    kvfull[:, jt * 2 * N + N:(jt + 1) * 2 * N],
                    at_sb[:, jt * 256:(jt + 1) * 256],
                    start=(jt == 0), stop=(jt == 15))

            # rinv_row [1, 256] via PE transposes, halved, then rank-1
            # broadcast down 48 spatial rows
            ps_r = ptrans.tile([128, 256], F32, tag="ptrans", name="ps_r")
            for m in range(2):
                nc.tensor.transpose(
                    ps_r[0:1, m * 128:(m + 1) * 128], rinvh[m][1][:],
                    ident[:])
            rrow = pers.tile([128, OLOC], F32, tag="rrow")
            nc.scalar.copy(rrow[0:1, :], ps_r[0:1, :])
            bcast = ptrans.tile([128, 256], F32, tag="ptrans", name="bcast")
            nc.tensor.matmul(
                bcast[0:N, :], ones1[0:1, 0:N], rrow[0:1, :],
                start=True, stop=True)
            bcast_sb = pers.tile([128, OLOC], F32, tag="bcast_sb")
            nc.scalar.copy(bcast_sb[0:N, :], bcast[0:N, :])
            t1s = pers.tile([128, OLOC], F32, tag="t1s")
            nc.vector.tensor_tensor(
                t1s[0:N, :], psT1[0:N, :], bcast_sb[0:N, :],
                op=mybir.AluOpType.mult)

            # post-RS: one fused vector op + a contiguous store
            rst = pers.tile([128, OLOC], BF16, tag="rst")
            nc.gpsimd.dma_start(out=rst[0:N, :], in_=rsbt[:])
            fin = pers.tile([128, OLOC], F32, tag="fin")
            nc.vector.scalar_tensor_tensor(
                out=fin[0:N, :], in0=rst[0:N, :], scalar=0.5,
                in1=t1s[0:N, :],
                op0=mybir.AluOpType.mult, op1=mybir.AluOpType.add)
            nc.scalar.dma_start(out=out_d[:], in_=fin[0:N, :])

    return nc


_NC_CACHE = None


def _get_nc():
    global _NC_CACHE
    if _NC_CACHE is None:
        _NC_CACHE = _build_nc()
    return _NC_CACHE


def _prep_w(W: np.ndarray) -> list[np.ndarray]:
    """Per-core streaming slab: [NCHUNK, 128, CHUNK_KT*OCOLS] bf16.

    col = (b*2+r)*256 + o_loc ; k-tile (t, a) row p holds channel
    i = p*16 + a (matches the contiguous per-partition Z DMA layout).
    """
    import ml_dtypes

    def sweep(wt, groups, cols, nch, ck):
        blk = np.concatenate(
            [wt[:, g * OLOC:(g + 1) * OLOC] for g in groups], axis=1)
        blk = blk.reshape(nch, ck, 128, cols).transpose(0, 2, 1, 3)
        return np.ascontiguousarray(
            blk.reshape(nch, 128, ck * cols).astype(ml_dtypes.bfloat16))

    shards = []
    for c in range(CORES):
        wc = W[:, :, c * OLOC:(c + 1) * OLOC, :, :]        # [3,2,256,2048,9]
        wt = wc.transpose(4, 3, 0, 1, 2).reshape(KTOT, OCOLS)
        # permute k rows: i = (p, a) -> tile row order (a, p)
        wt = wt.reshape(KD, 128, ISUB, OCOLS).transpose(0, 2, 1, 3)
        wt = np.ascontiguousarray(wt.reshape(KTOT, OCOLS))
        shards.append((sweep(wt, [2, 4, 3, 5], ACOLS, NCHA, CKA),
                       sweep(wt, [0, 1], BCOLS, NCHB, CKB)))
    return shards


def kernel(Z: np.ndarray, L: np.ndarray, W: np.ndarray) -> np.ndarray:
    nc = _get_nc()
    wts = _prep_w(np.asarray(W, dtype=np.float32))
    z = np.ascontiguousarray(Z, dtype=np.float32)
    l = np.ascontiguousarray(L, dtype=np.float32)
    in_maps = [{"wta": wts[c][0], "wtb": wts[c][1], "z": z, "l": l}
               for c in range(CORES)]
    trace = bool(int(os.environ.get("KERNEL_TRACE", "0")))
    kw = {}
    if trace and int(os.environ.get("KERNEL_TRACE_ALL", "0")):
        kw["trace_cores"] = list(range(CORES))
    res = run_bass_kernel_spmd(nc, in_maps, list(range(CORES)), trace=trace, **kw)
    kernel.last_result = res
    out = np.concatenate(
        [np.ascontiguousarray(res.results[c]["out"].T) for c in range(CORES)],
        axis=0)
    return out



# revision 54
# speedup vs baseline: 1.5800x; 1.5800x over previous
"""Trainium2 Bass kernel for nn_Attention_25915832664752.

Reference computation (per reference.py):
    For b in {Q,K,V}:  q0 = relu(IN(conv1d(Z, W[b,0])));  q1 = relu(IN(conv1d(Z, W[b,1]) @ L))
                       X_b = q0 + q1                                  [2048, 48]
    A  = exp(Q @ K^T)                                                 [2048, 2048]
    P  = A / rowsum(A);  Aa = (P + P^T)/2;  out = Aa @ V              [2048, 48]

Strategy (8 NeuronCores, tensor-parallel over nhid):
    Core c owns output channels [c*256, (c+1)*256).  W is pre-transposed on the
    host into a per-core streaming slab Wt[kt, p, o] with contraction index
    k on the partition axis, so the conv becomes a pure stream of [128, 48]
    stationary (shifted Z window) x W^T-column moving matmuls accumulated in
    PSUM — W (56 MB/core bf16) is read from HBM exactly once at full rate.
    BOTH conv branches share the same Z-window stationary: the "@ L" of the
    r=1 branch is applied at the epilogue, fused into the PSUM->IN-layout
    transpose (multiply by L instead of the identity — zero extra matmuls).
    After the convs: instance-norm + relu fused into one scalar-engine
    activation per tile; K and V are all-gathered; each core computes its
    row-block A_loc = exp(Q_loc K_full^T) and the transposed block
    At = exp(K_full Q_loc^T); then
        out = 0.5*rinv*(At^T-contract V_full) + 0.5*ReduceScatter(A_loc^T
              row-scaled V_loc)
    which realizes the symmetrized row-normalized attention exactly; the
    At/term1 compute hides under the ReduceScatter.
"""

import os
import sys

import numpy as np

sys.path.insert(0, "/opt/trn_rl_repo")

import orjson

import concourse.bass as bass
import concourse.mybir as mybir
from concourse import masks, tile
from concourse.bass_utils import run_bass_kernel_spmd

# ---------------------------------------------------------------- waitfix ---
# This neuronxcc build allows only ONE sync wait per instruction;
# TileContext emits instructions with several.  Rewrite the serialized BIR:
# hoist extra waits onto standalone NoOps inserted just before the
# instruction on the same engine (cumulative thresholds -> semantics kept).

_DMA_OPCODES = {
    "DMACopy", "DMATranspose", "TensorLoad", "TensorSave",
    "TriggeredCopy", "CollectiveCompute",
}
_wfix_counter = [0]


def _fix_block(instructions):
    out = []
    for ins in instructions:
        si = ins.get("sync_info")
        if not si:
            out.append(ins)
            continue
        waits = si.get("on_wait") or []
        updates = si.get("on_update") or []
        if len(waits) > 1:
            for w in waits[1:]:
                _wfix_counter[0] += 1
                out.append({
                    "engine": ins["engine"], "ins": [],
                    "name": f"WFIX-{_wfix_counter[0]}", "opcode": "NoOp",
                    "outs": [],
                    "sync_info": {"on_update": [], "on_wait": [w]},
                })
            si["on_wait"] = waits[:1]
        deferred = []
        if len(updates) > 1:
            assert ins.get("opcode", "") not in _DMA_OPCODES, (
                f"multi-update on DMA opcode: {ins['name']}"
            )
            si["on_update"] = updates[:1]
            for u in updates[1:]:
                _wfix_counter[0] += 1
                deferred.append({
                    "engine": ins["engine"], "ins": [],
                    "name": f"WFIX-{_wfix_counter[0]}", "opcode": "NoOp",
                    "outs": [],
                    "sync_info": {"on_update": [u], "on_wait": []},
                })
        out.append(ins)
        out.extend(deferred)
    return out


def _fix_bir_json_bytes(data: bytes) -> bytes:
    d = orjson.loads(data)
    for func in d.get("functions", []):
        for bb in func.get("blocks", []):
            bb["instructions"] = _fix_block(bb["instructions"])
    return orjson.dumps(d)


if not getattr(bass.Bass, "_waitfix_installed", False):
    _orig_to_json_bytes = bass.Bass.to_json_bytes

    def _patched_to_json_bytes(self) -> bytes:
        return _fix_bir_json_bytes(_orig_to_json_bytes(self))

    bass.Bass.to_json_bytes = _patched_to_json_bytes
    bass.Bass._waitfix_installed = True

# Synthesize the missing ``antenv.axon_hooks`` module so that
# ``run_bass_kernel_spmd(trace=True)`` can drive NTFF profiling through the
# axon PJRT plugin (the boot-time registration degrades silently when the
# module is absent).  Harmless when tracing is never requested.
try:
    import types

    import antenv

    if not hasattr(antenv, "axon_hooks"):
        _hooks_mod = types.ModuleType("antenv.axon_hooks")
        _ntff_hook = [None]
        _hooks_mod.set_axon_ntff_profile_hook = lambda h: _ntff_hook.__setitem__(0, h)
        _hooks_mod.get_axon_ntff_profile_hook = lambda: _ntff_hook[0]
        sys.modules["antenv.axon_hooks"] = _hooks_mod
        antenv.axon_hooks = _hooks_mod
        from trn_agent_boot.trn_boot import _ntff_profile_via_ctypes

        _hooks_mod.set_axon_ntff_profile_hook(
            _ntff_profile_via_ctypes("/opt/axon/libaxon_pjrt.so"))

    import concourse.bass_utils as _bu

    _bu.upload_artifacts = lambda tmpdir: tmpdir  # no fish share in container
except Exception:  # pragma: no cover - profiling is best-effort
    pass

# ------------------------------------------------------------- constants ---

NHID = 2048
NOPEN = 2048
N = 48          # spatial length
KD = 9          # conv kernel width
PAD = 4
NP = N + 2 * PAD            # 56 padded spatial
EPS = 1e-5
CORES = 8
OLOC = NHID // CORES        # 256 output channels per core
NGRP = 6                    # (b, r) conv groups
OCOLS = NGRP * OLOC         # 1536 W^T columns per core
KTOT = KD * NOPEN           # 18432 contraction length
NKT = KTOT // 128           # 144 k-tiles
ISUB = NOPEN // 128         # 16 i-subtiles
CKA = 4                     # k-tiles per W DMA chunk (sweep A, 1 MB bf16)
NCHA = NKT // CKA           # 36 chunks (sweep A)
CKB = 8                     # k-tiles per W DMA chunk (sweep B, 1 MB bf16)
NCHB = NKT // CKB           # 18 chunks (sweep B)
ACOLS = 4 * OLOC            # sweep A (Q,K): 1024 W^T cols per k-row
BCOLS = 2 * OLOC            # sweep B (V):    512 W^T cols per k-row
F32 = mybir.dt.float32
F32R = mybir.dt.float32r
BF16 = mybir.dt.bfloat16


DEBUG = bool(int(os.environ.get("KERNEL_DEBUG", "0")))


def _build_nc():
    nc = bass.Bass()

    wta_d = nc.declare_dram_parameter(
        "wta", [NCHA, 128, CKA * ACOLS], BF16, isOutput=False)
    wtb_d = nc.declare_dram_parameter(
        "wtb", [NCHB, 128, CKB * BCOLS], BF16, isOutput=False)
    z_d = nc.declare_dram_parameter("z", [NOPEN, N], F32, isOutput=False)
    l_d = nc.declare_dram_parameter("l", [N, N], F32, isOutput=False)
    out_d = nc.declare_dram_parameter("out", [N, OLOC], F32, isOutput=True)
    if DEBUG:
        dbg_conv = nc.declare_dram_parameter(
            "dbg_conv", [12, 128, N], F32, isOutput=True)
        dbg_qkv = nc.declare_dram_parameter(
            "dbg_qkv", [3, 2, 128, N], F32, isOutput=True)
        dbg_rs = nc.declare_dram_parameter(
            "dbg_rs", [2, 128, 1], F32, isOutput=True)

    with tile.TileContext(nc) as tc:
        with (
            tc.tile_pool(name="pers", bufs=1) as pers,
            tc.tile_pool(name="wpool", bufs=4) as wpool,
            tc.tile_pool(name="wpoolb", bufs=3) as wpoolb,
            tc.tile_pool(name="stats", bufs=1) as stats,
            tc.tile_pool(name="pacc", bufs=2, space="PSUM") as pacc,
            tc.tile_pool(name="ptrans", bufs=2, space="PSUM") as ptrans,
            tc.tile_pool(name="dram", bufs=1, space="DRAM") as dram,
        ):
            # ---------------- prologue: Z, L, identity, ZpadT, ZcolL -------
            ident = pers.tile([128, 128], F32, tag="ident")
            masks.make_identity(nc, ident[:])
            ident16 = pers.tile([128, 128], BF16, tag="ident16")
            nc.vector.tensor_copy(ident16[:], ident[:])

            # preload the Exp activation table now so the mid-kernel EXP
            # doesn't eat an ACT_TABLE_LOAD on the critical tail
            warm = pers.tile([128, 1], F32, tag="warm")
            nc.scalar.activation(warm[0:1, 0:1], ident[0:1, 0:1],
                                 mybir.ActivationFunctionType.Exp)
            ones1 = pers.tile([128, N], F32, tag="ones1")
            nc.vector.memset(ones1[0:1, :], 1.0)

            rg = [list(range(CORES))]

            # First two W chunks DMA before anything else so the conv can
            # start the moment zpadr is ready
            wts_pre = {}
            for g in (0, 1):
                wt0 = wpool.tile([128, CKA * ACOLS], BF16, tag="wt",
                                 name=f"wta{g}")
                nc.sync.dma_start(out=wt0[:], in_=wta_d[g])
                wts_pre[g] = wt0

            # Z loaded contiguously (channel i = p*16 + a: one 3 KB
            # descriptor per partition; _prep_w permutes W's k-rows to
            # match), then ONE strided vector cast-copy into the padded
            # bf16 conv-stationary layout: 16 tiles [128, 56] side by side.
            ztmp = pers.tile([128, ISUB * N], F32, tag="ztmp")
            nc.sync.dma_start(
                out=ztmp[:],
                in_=z_d[:].rearrange("(p a) n -> p (a n)", p=128))

            # L [48, 48] — used at the conv epilogues to apply the r=1
            # branch's "@ L" fused with the IN-layout transpose
            l_sb = pers.tile([128, N], F32, tag="l_sb")
            nc.sync.dma_start(out=l_sb[0:N, :], in_=l_d[:])

            zpadr = pers.tile([128, ISUB * NP], BF16, tag="zpadr")
            nc.vector.memset(zpadr[:], 0.0)
            zpr_v = zpadr[:].rearrange("p (a c) -> p a c", c=NP)
            nc.vector.tensor_copy(
                zpr_v[:, :, PAD:PAD + N],
                ztmp[:].rearrange("p (a n) -> p a n", n=N))

            # ---------------- conv: stream W as the MOVING operand ---------
            # lhsT (stationary) = [128, 48] shifted Z window, shared by BOTH
            # branch accumulators; rhs = W^T columns streaming at 1 col/cycle.
            # Two k-sweeps: A covers K+V groups, B covers Q, so the K/V
            # all-gathers hide behind sweep B.  One PSUM bank per branch
            # accumulator (start=True clears has_written for the whole bank).
            relu_sc = pers.tile([128, 12 * N], F32, tag="relu_sc")
            yt_sb = pers.tile([128, 6 * OLOC], F32, tag="yt_sb")
            qkv = [pers.tile([128, 2 * N], F32, tag=f"qkv{b}", name=f"qkv{b}")
                   for b in range(3)]
            slotinfo = {}

            def sweep_epilogue(entries, label):
                """entries: list of (g, acc_ap[48, 256], is_r1).  Transpose
                each half to [128, 48] — for r=1 groups multiply by L instead
                of the identity, realizing (conv @ L)^T in the same matmul —
                then batched instance-norm stats (one vector op per stage
                across all slots) + fused relu."""
                nslot = 2 * len(entries)
                xc = stats.tile([128, nslot * N], F32, tag=f"xc{label}",
                                name=f"xc{label}")
                slots = []
                for idx, (g, acc_ap, is_r1) in enumerate(entries):
                    nc.scalar.copy(
                        yt_sb[0:N, g * OLOC:(g + 1) * OLOC], acc_ap)
                    rmat = l_sb if is_r1 else ident
                    for h in range(2):
                        ot = g * 2 + h
                        slot = idx * 2 + h
                        ps2 = ptrans.tile([128, 128], F32, tag="ptrans",
                                          name=f"tp{ot}")
                        nc.tensor.matmul(
                            ps2[:, 0:N],
                            yt_sb[0:N, g * OLOC + h * 128:
                                  g * OLOC + (h + 1) * 128],
                            rmat[0:N, 0:N],
                            start=True, stop=True)
                        nc.scalar.copy(xc[:, slot * N:(slot + 1) * N],
                                       ps2[:, 0:N])
                        if DEBUG:
                            nc.scalar.dma_start(
                                out=dbg_conv[ot],
                                in_=xc[:, slot * N:(slot + 1) * N])
                        slots.append((ot, slot))
                sm = stats.tile([128, nslot], F32, tag=f"sm{label}",
                                name=f"sm{label}")
                sq = stats.tile([128, nslot], F32, tag=f"sq{label}",
                                name=f"sq{label}")
                scr = stats.tile([128, nslot * N], F32, tag=f"scr{label}",
                                 name=f"scr{label}")
                for ot, slot in slots:
                    nc.vector.reduce_sum(
                        sm[:, slot:slot + 1], xc[:, slot * N:(slot + 1) * N],
                        axis=mybir.AxisListType.X)
                nc.vector.tensor_tensor(scr[:], xc[:], xc[:],
                                        op=mybir.AluOpType.mult)
                for ot, slot in slots:
                    nc.vector.reduce_sum(
                        sq[:, slot:slot + 1], scr[:, slot * N:(slot + 1) * N],
                        axis=mybir.AxisListType.X)
                mean = stats.tile([128, nslot], F32, tag=f"mean{label}",
                                  name=f"mean{label}")
                var = stats.tile([128, nslot], F32, tag=f"var{label}",
                                 name=f"var{label}")
                std = stats.tile([128, nslot], F32, tag=f"std{label}",
                                 name=f"std{label}")
                rsv = stats.tile([128, nslot], F32, tag=f"rsv{label}",
                                 name=f"rsv{label}")
                nc.vector.tensor_scalar_mul(mean[:], sm[:], 1.0 / N)
                nc.vector.tensor_scalar_mul(sq[:], sq[:], 1.0 / N)
                nc.vector.tensor_tensor(var[:], mean[:], mean[:],
                                        op=mybir.AluOpType.mult)
                nc.vector.tensor_tensor(var[:], sq[:], var[:],
                                        op=mybir.AluOpType.subtract)
                nc.vector.tensor_scalar_add(var[:], var[:], EPS)
                nc.scalar.sqrt(std[:], var[:])
                nc.vector.reciprocal(rsv[:], std[:])
                if label == "B":
                    # preload the Exp table while the vector engine runs the
                    # IN+relu tail, so the attention EXPs start cold-free
                    nc.scalar.activation(warm[0:1, 0:1], ident[0:1, 0:1],
                                         mybir.ActivationFunctionType.Exp)
                for ot, slot in slots:
                    slotinfo[ot] = (xc, scr, slot, mean, rsv)

            def qkv_add(b):
                # IN + relu + branch-add fused on the vector engine:
                #   q = rsv0*max(x0-m0, 0) + rsv1*max(x1-m1, 0)
                for h in range(2):
                    ot0 = (2 * b) * 2 + h        # r = 0
                    ot1 = (2 * b + 1) * 2 + h    # r = 1
                    xc0, scr0, s0, mean0, rsv0 = slotinfo[ot0]
                    xc1, scr1, s1, mean1, rsv1 = slotinfo[ot1]
                    nc.vector.tensor_scalar(
                        scr0[:, s0 * N:(s0 + 1) * N],
                        xc0[:, s0 * N:(s0 + 1) * N],
                        mean0[:, s0:s0 + 1], 0.0,
                        op0=mybir.AluOpType.subtract, op1=mybir.AluOpType.max)
                    nc.vector.tensor_scalar(
                        scr1[:, s1 * N:(s1 + 1) * N],
                        xc1[:, s1 * N:(s1 + 1) * N],
                        mean1[:, s1:s1 + 1], 0.0,
                        op0=mybir.AluOpType.subtract, op1=mybir.AluOpType.max)
                    nc.vector.tensor_scalar_mul(
                        relu_sc[:, ot1 * N:(ot1 + 1) * N],
                        scr1[:, s1 * N:(s1 + 1) * N], rsv1[:, s1:s1 + 1])
                    nc.vector.scalar_tensor_tensor(
                        out=qkv[b][:, h * N:(h + 1) * N],
                        in0=scr0[:, s0 * N:(s0 + 1) * N],
                        scalar=rsv0[:, s0:s0 + 1],
                        in1=relu_sc[:, ot1 * N:(ot1 + 1) * N],
                        op0=mybir.AluOpType.mult,
                        op1=mybir.AluOpType.add)
                    if DEBUG:
                        nc.scalar.dma_start(
                            out=dbg_qkv[b, h],
                            in_=qkv[b][:, h * N:(h + 1) * N])

            # ---- sweep A: K + V (cols [g2,g4 | g3,g5]; both branch
            # accumulators share one Z-window stationary per k-tile)
            accA = [pacc.tile([128, 2 * OLOC], F32, tag="accw", name=f"accA{i}")
                    for i in range(2)]
            for gch in range(NCHA):
                if gch in wts_pre:
                    wt = wts_pre.pop(gch)
                else:
                    wt = wpool.tile([128, CKA * ACOLS], BF16, tag="wt",
                                    name=f"wta{gch}")
                    nc.sync.dma_start(out=wt[:], in_=wta_d[gch])
                for j in range(CKA):
                    kt = gch * CKA + j
                    t, s = kt // ISUB, kt % ISUB
                    lhs0 = zpadr[:, s * NP + t: s * NP + t + N]
                    base = j * ACOLS
                    nc.tensor.matmul(
                        accA[0][0:N, :], lhs0, wt[:, base: base + 512],
                        start=(kt == 0), stop=(kt == NKT - 1))
                    nc.tensor.matmul(
                        accA[1][0:N, :], lhs0, wt[:, base + 512: base + 1024],
                        start=(kt == 0), stop=(kt == NKT - 1))

            # K then V epilogues; ONE merged all-gather of [256, 96]
            # (K | V side by side) hides behind sweep B (Q)
            qloc, kloc, vloc = qkv
            kvb = dram.tile([OLOC, 2 * N], BF16, tag="kvb")
            kvg = dram.tile([NHID, 2 * N], BF16, tag="kvg",
                            addr_space="Shared")
            kv16 = pers.tile([128, 4 * N], BF16, tag="kv16")

            sweep_epilogue([(2, accA[0][0:N, 0:OLOC], False),
                            (3, accA[1][0:N, 0:OLOC], True)], "K")
            qkv_add(1)
            nc.vector.tensor_copy(kv16[:, 0:2 * N], kloc[:])

            sweep_epilogue([(4, accA[0][0:N, OLOC:2 * OLOC], False),
                            (5, accA[1][0:N, OLOC:2 * OLOC], True)], "V")
            qkv_add(2)
            nc.vector.tensor_copy(kv16[:, 2 * N:4 * N], vloc[:])
            # kv16 col blocks are [K0 K1 V0 V1]; kvb row h*128+p gets
            # [K_h | V_h] for channel h*128+p
            nc.scalar.dma_start(
                out=kvb[:, 0:N].rearrange("(h p) n -> p h n", h=2),
                in_=kv16[:, 0:2 * N].rearrange("p (h n) -> p h n", h=2))
            nc.scalar.dma_start(
                out=kvb[:, N:2 * N].rearrange("(h p) n -> p h n", h=2),
                in_=kv16[:, 2 * N:4 * N].rearrange("p (h n) -> p h n", h=2))
            nc.gpsimd.collective_compute(
                "AllGather", mybir.AluOpType.bypass,
                replica_groups=rg, ins=[kvb.opt()], outs=[kvg.opt()])
            kvfull = pers.tile([128, 16 * 2 * N], BF16, tag="kvfull")
            nc.gpsimd.dma_start(
                out=kvfull[:].rearrange("p (a c) -> p a c", c=2 * N),
                in_=kvg[:].rearrange("(a p) c -> p a c", p=128))

            # ---- sweep B: Q ----
            accB = [pacc.tile([128, OLOC], F32, tag="acc", name=f"accB{i}")
                    for i in range(2)]  # order: g0, g1

            def sweep_b(c0, c1):
                for gch in range(c0, c1):
                    wt = wpoolb.tile([128, CKB * BCOLS], BF16, tag="wtb",
                                     name=f"wtb{gch}")
                    nc.sync.dma_start(out=wt[:], in_=wtb_d[gch])
                    for j in range(CKB):
                        kt = gch * CKB + j
                        t, s = kt // ISUB, kt % ISUB
                        lhs0 = zpadr[:, s * NP + t: s * NP + t + N]
                        base = j * BCOLS
                        nc.tensor.matmul(
                            accB[0][0:N, :], lhs0,
                            wt[:, base: base + OLOC],
                            start=(kt == 0), stop=(kt == NKT - 1))
                        nc.tensor.matmul(
                            accB[1][0:N, :], lhs0,
                            wt[:, base + OLOC: base + 2 * OLOC],
                            start=(kt == 0), stop=(kt == NKT - 1))

            sweep_b(0, NCHB)

            sweep_epilogue([(0, accB[0][0:N, :], False),
                            (1, accB[1][0:N, :], True)], "B")
            qkv_add(0)

            # kT transposes (kvfull landed under sweep B; deliberately NOT
            # mid-sweep — under HBM contention the AG completion time is too
            # variable and a mid-sweep dependency stalls the whole PE)
            kT = pers.tile([128, NHID], BF16, tag="kT")
            for jt in range(16):
                ps = ptrans.tile([128, 128], F32, tag="ptrans")
                nc.tensor.matmul(
                    ps[0:N, :], kvfull[:, jt * 2 * N:jt * 2 * N + N],
                    ident16[:], start=True, stop=True)
                kt_copy = (nc.scalar.copy if jt % 2 == 0
                           else nc.vector.tensor_copy)
                kt_copy(kT[0:N, jt * 128:(jt + 1) * 128], ps[0:N, :])

            # qT split exactly into bf16 high + low parts: every attention
            # matmul runs with bf16 operands at full PE rate with no
            # precision loss vs f32 Q (K is bf16-limited by the all-gather).
            qTh = pers.tile([128, 2 * 128], BF16, tag="qTh")
            qTl = pers.tile([128, 2 * 128], BF16, tag="qTl")
            for h in range(2):
                ps = ptrans.tile([128, 128], F32, tag="ptrans")
                nc.tensor.transpose(
                    ps[0:N, :], qloc[:, h * N:(h + 1) * N], ident[:])
                nc.scalar.copy(qTh[0:N, h * 128:(h + 1) * 128], ps[0:N, :])
                nc.vector.tensor_tensor(
                    qTl[0:N, h * 128:(h + 1) * 128], ps[0:N, :],
                    qTh[0:N, h * 128:(h + 1) * 128],
                    op=mybir.AluOpType.subtract)

            # A = exp(Q K^T) chunks (kT already transposed mid-sweep)
            a_sb = [pers.tile([128, NHID], BF16, tag=f"a{m}", name=f"a{m}")
                    for m in range(2)]
            rsparts = [stats.tile([128, 4], F32, tag=f"rsp{m}", name=f"rsp{m}")
                       for m in range(2)]
            for jc in range(4):
                for m in range(2):
                    ps = ptrans.tile([128, 512], F32, tag="pattn",
                                     name=f"pa{m}{jc}", bufs=2)
                    nc.tensor.matmul(
                        ps[:, 0:512],
                        qTh[0:N, m * 128:(m + 1) * 128],
                        kT[0:N, jc * 512:(jc + 1) * 512],
                        start=True, stop=False)
                    nc.tensor.matmul(
                        ps[:, 0:512],
                        qTl[0:N, m * 128:(m + 1) * 128],
                        kT[0:N, jc * 512:(jc + 1) * 512],
                        start=False, stop=True)
                    nc.scalar.activation(
                        a_sb[m][:, jc * 512:(jc + 1) * 512], ps[:, 0:512],
                        mybir.ActivationFunctionType.Exp,
                        accum_out=rsparts[m][:, jc:jc + 1])
            rinvh = []
            for m in range(2):
                rowsum = stats.tile([128, 1], F32, tag=f"rowsum{m}", name=f"rowsum{m}")
                nc.vector.reduce_sum(rowsum[:], rsparts[m][:], axis=mybir.AxisListType.X)
                rinv = stats.tile([128, 1], F32, tag=f"rinv{m}", name=f"rinv{m}")
                nc.vector.reciprocal(rinv[:], rowsum[:])
                rh = stats.tile([128, 1], F32, tag=f"rinvh{m}", name=f"rinvh{m}")
                nc.vector.tensor_scalar_mul(rh[:], rinv[:], 0.5)
                rinvh.append((rinv, rh))
                if DEBUG:
                    nc.scalar.dma_start(out=dbg_rs[m], in_=rowsum[:])

            # ---------------- U^T = (rinv*V_loc)^T-contract A_loc ----------
            # two wide matmuls per 512-col chunk (vr stationary, bf16 a_sb
            # moving) replace 32 LDWEIGHTS-bound small matmuls; then PE
            # transposes back to [2048, 48] for the ReduceScatter.
            vrb = pers.tile([128, 2 * N], BF16, tag="vrb")
            for m in range(2):
                nc.vector.tensor_scalar_mul(
                    vrb[:, m * N:(m + 1) * N], vloc[:, m * N:(m + 1) * N],
                    rinvh[m][0][:])
            uT = pers.tile([128, NHID], BF16, tag="uT")
            for jc in range(4):
                ps = ptrans.tile([128, 512], F32, tag="pattn",
                                 name=f"pu{jc}", bufs=2)
                for m in range(2):
                    nc.tensor.matmul(
                        ps[0:N, :], vrb[:, m * N:(m + 1) * N],
                        a_sb[m][:, jc * 512:(jc + 1) * 512],
                        start=(m == 0), stop=(m == 1))
                nc.scalar.copy(uT[0:N, jc * 512:(jc + 1) * 512], ps[0:N, :])

            # U^T goes to the ReduceScatter in transposed block layout
            # [8, 48, 256] (block c = core c's channel slab): no PE
            # transposes before the collective; each core transposes only
            # its own [48, 256] result afterwards.  Eight contiguous
            # per-block DMAs spread across the engine queues (one strided
            # DMA would cost 384 scattered descriptors on one queue).
            ubt = dram.tile([CORES * N, OLOC], BF16, tag="ubt")
            rsbt = dram.tile([N, OLOC], BF16, tag="rsbt")
            ub_eng = [nc.sync, nc.scalar, nc.gpsimd]
            for c in range(CORES):
                ub_eng[c % 3].dma_start(
                    out=ubt[c * N:(c + 1) * N, :],
                    in_=uT[0:N, c * OLOC:(c + 1) * OLOC])
            nc.gpsimd.collective_compute(
                "ReduceScatter", mybir.AluOpType.add,
                replica_groups=rg, ins=[ubt.opt()], outs=[rsbt.opt()])

            # At = exp(K Q^T)  [2048, 256] — computed while the RS is on
            # the wire (bf16 kT stationary, bf16 qTh/qTl moving)
            at_sb = pers.tile([128, 16 * 256], BF16, tag="at")
            for jt in range(16):
                ps = ptrans.tile([128, 256], F32, tag="ptrans")
                nc.tensor.matmul(
                    ps[:, 0:256],
                    kT[0:N, jt * 128:(jt + 1) * 128],
                    qTh[0:N, 0:256],
                    start=True, stop=False)
                nc.tensor.matmul(
                    ps[:, 0:256],
                    kT[0:N, jt * 128:(jt + 1) * 128],
                    qTl[0:N, 0:256],
                    start=False, stop=True)
                nc.scalar.activation(
                    at_sb[:, jt * 256:(jt + 1) * 256], ps[:, 0:256],
                    mybir.ActivationFunctionType.Exp)

            # --------- term1^T = V_full^T-contract At (one accumulator),
            # pre-scaled by a PE-broadcast 0.5*rinv row — all while the RS
            # is on the wire.  The whole finale stays in [48, 256] layout;
            # the host transposes the gathered output once.
            psT1 = pacc.tile([128, OLOC], F32, tag="acc", name="pt1")
            for jt in range(16):
                nc.tensor.matmul(
                    psT1[0:N, :],
                    kvfull[:, jt * 2 * N + N:(jt + 1) * 2 * N],
                    at_sb[:, jt * 256:(jt + 1) * 256],
                    start=(jt == 0), stop=(jt == 15))

            # rinv_row [1, 256] via PE transposes, halved, then rank-1
            # broadcast down 48 spatial rows
            ps_r = ptrans.tile([128, 256], F32, tag="ptrans", name="ps_r")
            for m in range(2):
                nc.tensor.transpose(
                    ps_r[0:1, m * 128:(m + 1) * 128], rinvh[m][1][:],
                    ident[:])
            rrow = pers.tile([128, OLOC], F32, tag="rrow")
            nc.scalar.copy(rrow[0:1, :], ps_r[0:1, :])
            bcast = ptrans.tile([128, 256], F32, tag="ptrans", name="bcast")
            nc.tensor.matmul(
                bcast[0:N, :], ones1[0:1, 0:N], rrow[0:1, :],
                start=True, stop=True)
            bcast_sb = pers.tile([128, OLOC], F32, tag="bcast_sb")
            nc.scalar.copy(bcast_sb[0:N, :], bcast[0:N, :])
            t1s = pers.tile([128, OLOC], F32, tag="t1s")
            nc.vector.tensor_tensor(
                t1s[0:N, :], psT1[0:N, :], bcast_sb[0:N, :],
                op=mybir.AluOpType.mult)

            # post-RS: one fused vector op + a contiguous store
            rst = pers.tile([128, OLOC], BF16, tag="rst")
            nc.gpsimd.dma_start(out=rst[0:N, :], in_=rsbt[:])
            fin = pers.tile([128, OLOC], F32, tag="fin")
            nc.vector.scalar_tensor_tensor(
                out=fin[0:N, :], in0=rst[0:N, :], scalar=0.5,
                in1=t1s[0:N, :],
                op0=mybir.AluOpType.mult, op1=mybir.AluOpType.add)
            nc.scalar.dma_start(out=out_d[:], in_=fin[0:N, :])

    return nc


_NC_CACHE = None


def _get_nc():
    global _NC_CACHE
    if _NC_CACHE is None:
        _NC_CACHE = _build_nc()
    return _NC_CACHE


def _prep_w(W: np.ndarray) -> list[np.ndarray]:
    """Per-core streaming slab: [NCHUNK, 128, CHUNK_KT*OCOLS] bf16.

    col = (b*2+r)*256 + o_loc ; k-tile (t, a) row p holds channel
    i = p*16 + a (matches the contiguous per-partition Z DMA layout).
    """
    import ml_dtypes

    def sweep(wt, groups, cols, nch, ck):
        blk = np.concatenate(
            [wt[:, g * OLOC:(g + 1) * OLOC] for g in groups], axis=1)
        blk = blk.reshape(nch, ck, 128, cols).transpose(0, 2, 1, 3)
        return np.ascontiguousarray(
            blk.reshape(nch, 128, ck * cols).astype(ml_dtypes.bfloat16))

    shards = []
    for c in range(CORES):
        wc = W[:, :, c * OLOC:(c + 1) * OLOC, :, :]        # [3,2,256,2048,9]
        wt = wc.transpose(4, 3, 0, 1, 2).reshape(KTOT, OCOLS)
        # permute k rows: i = (p, a) -> tile row order (a, p)
        wt = wt.reshape(KD, 128, ISUB, OCOLS).transpose(0, 2, 1, 3)
        wt = np.ascontiguousarray(wt.reshape(KTOT, OCOLS))
        shards.append((sweep(wt, [2, 4, 3, 5], ACOLS, NCHA, CKA),
                       sweep(wt, [0, 1], BCOLS, NCHB, CKB)))
    return shards


def kernel(Z: np.ndarray, L: np.ndarray, W: np.ndarray) -> np.ndarray:
    nc = _get_nc()
    wts = _prep_w(np.asarray(W, dtype=np.float32))
    z = np.ascontiguousarray(Z, dtype=np.float32)
    l = np.ascontiguousarray(L, dtype=np.float32)
    in_maps = [{"wta": wts[c][0], "wtb": wts[c][1], "z": z, "l": l}
               for c in range(CORES)]
    trace = bool(int(os.environ.get("KERNEL_TRACE", "0")))
    kw = {}
    if trace and int(os.environ.get("KERNEL_TRACE_ALL", "0")):
        kw["trace_cores"] = list(range(CORES))
    res = run_bass_kernel_spmd(nc, in_maps, list(range(CORES)), trace=trace, **kw)
    kernel.last_result = res
    out = np.concatenate(
        [np.ascontiguousarray(res.results[c]["out"].T) for c in range(CORES)],
        axis=0)
    return out

